# revision 11
# baseline (speedup 1.0000x reference)
"""Multi-head self-attention on 8 Trainium2 NeuronCores.

Problem: x[4, 2048, 1024], 16 heads x 64 dims, fused qkv + attention + out-proj.

Sharding (hybrid, per the tensor-parallel hint): core c handles batch b = c//2
and head-group g = c%2 (8 of the 16 heads). Each core computes a partial
out-projection over its 8 heads; the host sums the two group partials per
batch and adds b_out (+ the folded v-bias term, see below).

The kernel is ACT(exp)-bound: 256 exp tiles of [128, 1024] at ~1.15us each
(~294us of scalar-engine work). Everything else is scheduled around keeping
ACT saturated from ~5us onward:
  - scores computed transposed (S^T[k, q]) per 128-row k-chunk; two heads of
    a pair row-packed on the PE (K=64 each) into one [128, 1024] PSUM tile
    that a single exp covers; exp reads PSUM, writes bf16 E to SBUF.
  - softmax denominator comes free as an all-ones column appended to V in
    the AV matmul (row 64 of the PSUM result).
  - normalization without ACT: PE rank-1 broadcast of the bf16 denominators,
    one DVE reciprocal_approx_fast on the [128, 512] broadcast (reads PSUM),
    then two DVE muls -> normalized waT.  No Ln/Exp table work.
  - v-bias folded out on the host: softmax weights sum to 1 exactly (the
    denominator IS the ones-column sum), so the bias contributes bv @ w_out,
    added to b_out host-side.  q/k biases stay in the projection.
  - emission is software-pipelined: the scores->exp->AV stream is the
    backbone (unit u's AV emitted during unit u+1); qkproj / vproj /
    out-proj chains are deadline-scheduled filler inside the stream, and
    input DMAs are sliced so the first scores matmul can start ~4us in.
"""

import os
import sys
from contextlib import ExitStack

import numpy as np

for _p in ("/opt/trn_rl_repo",):
    if _p not in sys.path and os.path.isdir(_p):
        sys.path.insert(0, _p)

import ml_dtypes

import concourse.bass as bass
import concourse.tile as tile
from concourse import bacc, mybir
from concourse.bass_utils import run_bass_kernel_spmd

BF16 = ml_dtypes.bfloat16
F32 = np.float32

D = 1024
H = 16
HD = 64
B = 4
N = 2048
NCORES = 8
G = 2  # head groups (tensor-parallel axis)
LH = H // G  # local heads per core
DC = D // 128  # 8 contraction chunks
KC = N // 128  # 16 k-token chunks
QT = N // 512  # 4 q tiles
TOK = N // 128  # 16 token chunks
NU = 16  # pipeline units: u = q4*4 + pair

_CACHE = {}


def _pin_act_tables():
    """Pin the act-table chooser so exp resolves to one stable set (no
    mid-kernel table reloads)."""
    if _CACHE.get("act_pinned"):
        return
    from concourse import bacc as _bacc
    from concourse import hw_specs as _hw

    orig = _hw.get_activation_tables

    def patched(arch):
        t = dict(orig(arch))
        keep = "natural_log_exp_and_others"
        if keep in t:
            pinned = t[keep]
            t = {n: (s if n == keep else (s - pinned)) for n, s in t.items()}
        return t

    _hw.get_activation_tables = patched
    _bacc.get_activation_tables = patched
    _CACHE["act_pinned"] = True


def _build_nc():
    _pin_act_tables()
    nc = bacc.Bacc(None, target_bir_lowering=False)

    xT = nc.declare_dram_parameter("xT", [128, DC, N], mybir.dt.bfloat16, isOutput=False)
    wqk = nc.declare_dram_parameter("wqk", [128, DC, 2 * LH * HD], mybir.dt.bfloat16, isOutput=False)
    bqk = nc.declare_dram_parameter("bqk", [128, DC], mybir.dt.float32, isOutput=False)
    wv = nc.declare_dram_parameter("wv", [128, DC, LH * HD], mybir.dt.bfloat16, isOutput=False)
    wout = nc.declare_dram_parameter("wout", [128, LH * HD // 128, D], mybir.dt.bfloat16, isOutput=False)
    out = nc.declare_dram_parameter("out", [N, D], mybir.dt.bfloat16, isOutput=True)

    with tile.TileContext(nc) as tc, ExitStack() as ctx:
        const = ctx.enter_context(tc.tile_pool(name="const", bufs=1))
        xpool = ctx.enter_context(tc.tile_pool(name="xpool", bufs=1))
        epool = ctx.enter_context(tc.tile_pool(name="epool", bufs=2))
        work = ctx.enter_context(tc.tile_pool(name="work", bufs=1))
        outp = ctx.enter_context(tc.tile_pool(name="outp", bufs=2))
        small = ctx.enter_context(tc.tile_pool(name="small", bufs=2))
        ps_s = ctx.enter_context(tc.tile_pool(name="ps_s", bufs=2, space="PSUM"))
        ps_wa = ctx.enter_context(tc.tile_pool(name="ps_wa", bufs=2, space="PSUM"))
        ps_m = ctx.enter_context(tc.tile_pool(name="ps_m", bufs=2, space="PSUM"))

        wqk_sb = const.tile([128, DC, 2 * LH * HD], mybir.dt.bfloat16)
        bqk_sb = const.tile([128, DC], mybir.dt.float32)
        wv_sb = const.tile([128, DC, LH * HD], mybir.dt.bfloat16)
        wout_sb = const.tile([128, LH * HD // 128, D], mybir.dt.bfloat16)
        ones_bf = const.tile([1, 128], mybir.dt.bfloat16)
        xT_sb = xpool.tile([128, DC, N], mybir.dt.bfloat16, tag="xT")
        qkT_sb = work.tile([128, DC, N], mybir.dt.bfloat16, tag="qkT")
        V_sb = work.tile([128, KC, LH, HD + 1], mybir.dt.bfloat16, tag="V")
        waT_sb = work.tile([128, LH * HD // 128, N], mybir.dt.bfloat16, tag="waT")

        # --- DMA: sliced so the first qk chains can start early ------------
        def dma_x(tt, chunked=False):
            # tt0 per contraction-chunk (warm-up pacers key off each chunk);
            # later slices as one descriptor to cut Sync issue serialization
            if chunked:
                for kc in range(DC):
                    nc.sync.dma_start(
                        out=xT_sb[:, kc, tt * 512 : (tt + 1) * 512],
                        in_=xT[:, kc, tt * 512 : (tt + 1) * 512],
                    )
            else:
                nc.sync.dma_start(
                    out=xT_sb[:, :, tt * 512 : (tt + 1) * 512],
                    in_=xT[:, :, tt * 512 : (tt + 1) * 512],
                )

        def dma_wqk(p):
            nc.sync.dma_start(
                out=wqk_sb[:, :, p * 256 : (p + 1) * 256],
                in_=wqk[:, :, p * 256 : (p + 1) * 256],
            )

        dma_wqk(0)
        nc.sync.dma_start(out=bqk_sb[:], in_=bqk[:])
        dma_x(0, chunked=True)
        nc.sync.dma_start(out=wv_sb[:], in_=wv[:])
        for tt in range(1, QT):
            dma_x(tt)
        for p in range(1, LH // 2):
            dma_wqk(p)
        nc.sync.dma_start(out=wout_sb[:], in_=wout[:])

        # earliest slot (in kc units, ~1.15us each) each DMA'd tensor is
        # usable; gates the budget puller so a DMA-blocked chain never
        # head-of-line-stalls the PE queue.
        arr_x = [0, 4, 6, 8]
        arr_wqk = [0, 10, 11, 12]
        ARR_WV = 2

        nc.vector.memset(ones_bf[:], 1.0)
        # only the ones column (index HD) of V needs initializing -- it feeds
        # the free softmax denominator; v-proj fills [0:HD].
        nc.vector.memset(V_sb[:, :, :, HD : HD + 1], 1.0)

        # HAM warm-up: a burst of junk matmuls unthrottles the PE clock
        # (~3.4us busy window), then one pacer MM per arriving x-chunk keeps
        # it warm across the DMA lead-in. Results go to a scratch PSUM tile
        # in the scores ring; nothing reads them.
        warm_ps = ps_s.tile([128, 1024], mybir.dt.float32, tag="sc", name="warm_ps")
        for i in range(40):
            nc.tensor.matmul(
                warm_ps[0:128, 0:64],
                lhsT=ones_bf[0:1, 0:128],
                rhs=ones_bf[0:1, 0:64],
                start=True,
                stop=True,
            )
        for kc in range(DC):
            nc.tensor.matmul(
                warm_ps[0:64, 0:64],
                lhsT=xT_sb[:, kc, 0:64],
                rhs=xT_sb[:, kc, 0:64],
                start=True,
                stop=True,
            )

        # --- filler work items (deadline-scheduled PE chains) --------------
        def emit_qk_chain(m, tt):
            pq = ps_m.tile([128, 512], mybir.dt.float32, tag="misc", name=f"pq_{m}_{tt}")
            for kc in range(DC):
                nc.tensor.matmul(
                    pq[:],
                    lhsT=wqk_sb[:, kc, m * 128 : (m + 1) * 128],
                    rhs=xT_sb[:, kc, tt * 512 : (tt + 1) * 512],
                    start=(kc == 0),
                    stop=(kc == DC - 1),
                )
            nc.vector.tensor_scalar_add(
                out=qkT_sb[:, m, tt * 512 : (tt + 1) * 512],
                in0=pq[:],
                scalar1=bqk_sb[:, m : m + 1],
            )

        def emit_v_chain(c):
            pv = ps_m.tile([128, 512], mybir.dt.float32, tag="misc", name=f"pv_{c}")
            for kc in range(DC):
                nc.tensor.matmul(
                    pv[:],
                    lhsT=xT_sb[:, kc, c * 128 : (c + 1) * 128],
                    rhs=wv_sb[:, kc, :],
                    start=(kc == 0),
                    stop=(kc == DC - 1),
                )
            nc.vector.tensor_copy(
                out=V_sb[:, c, :, 0:HD],
                in_=pv[:].rearrange("p (l d) -> p l d", l=LH),
            )

        def emit_out_chain(c):
            # k4-outer with both half-D accumulators open: 8 same-shape MMs
            # back-to-back (each lhsT used for both halves)
            po = [
                ps_m.tile([128, 512], mybir.dt.float32, tag="misc", name=f"po_{c}_{h}")
                for h in range(2)
            ]
            for k4 in range(LH * HD // 128):
                for half in range(2):
                    nc.tensor.matmul(
                        po[half][:],
                        lhsT=waT_sb[:, k4, c * 128 : (c + 1) * 128],
                        rhs=wout_sb[:, k4, half * 512 : (half + 1) * 512],
                        start=(k4 == 0),
                        stop=(k4 == LH * HD // 128 - 1),
                    )
            for half in range(2):
                o_sb = outp.tile([128, 512], mybir.dt.bfloat16, tag="osb", name=f"o_{c}_{half}")
                nc.vector.tensor_copy(out=o_sb[:], in_=po[half][:])
                nc.sync.dma_start(
                    out=out[c * 128 : (c + 1) * 128, half * 512 : (half + 1) * 512],
                    in_=o_sb[:],
                )

        # items: (deadline_g, seq, earliest_g, cost_ns, fn); deadline/earliest
        # in kc-slot units g = u*16 + kc
        fill = []
        seq = 0
        for p in range(LH // 2):
            for tt in range(QT):
                e = max(arr_x[tt], arr_wqk[p])
                fill.append((p * 16 + 4 * tt, seq, e, 1750,
                             (lambda m=2 * p + 1, t=tt: emit_qk_chain(m, t))))
                seq += 1
                fill.append(((4 * tt + p) * 16, seq, e, 1750,
                             (lambda m=2 * p, t=tt: emit_qk_chain(m, t))))
                seq += 1
        for c in range(TOK):
            fill.append((max(6 + c, arr_x[c // 4] + 1), seq,
                         max(arr_x[c // 4], ARR_WV), 1750,
                         (lambda cc=c: emit_v_chain(cc))))
            seq += 1
        fill.sort()
        from collections import deque

        fq = deque(fill)

        SLOT_NS = 1150.0  # ACT pace per kc
        debt = [0.0]

        def flush(g):
            while fq and fq[0][0] <= g:
                _, _, _, cns, fn = fq.popleft()
                fn()
                debt[0] += cns

        def pull(g):
            # pop DMA-ready items while the PE has slack
            while debt[0] < 300.0 and fq:
                if fq[0][2] > g:
                    break
                _, _, _, cns, fn = fq.popleft()
                fn()
                debt[0] += cns
            if debt[0] < -20000.0:
                debt[0] = -20000.0

        # --- the pipelined attention stream --------------------------------
        E_t = [None] * NU
        pw_t = [None] * NU

        def emit_norm(u):
            """Normalize unit u's AV output into waT (no ACT work): gather the
            two denominator rows, one rank-1 broadcast matmul into a scores-
            ring PSUM tile, one DVE reciprocal, two DVE muls."""
            q4u, pu = divmod(u, 4)
            q0 = q4u * 512
            den2 = small.tile([1, 1024], mybir.dt.bfloat16, tag="den", name=f"den_{u}")
            nc.vector.tensor_copy(out=den2[0:1, 0:512], in_=pw_t[u][0][64:65, :])
            nc.vector.tensor_copy(out=den2[0:1, 512:1024], in_=pw_t[u][1][64:65, :])
            pb = ps_m.tile([128, 512], mybir.dt.float32, tag="misc", name=f"pb_{u}")
            for h in range(2):
                nc.tensor.matmul(
                    pb[64 * h : 64 * h + 64, :],
                    lhsT=ones_bf[0:1, 0:64],
                    rhs=den2[0:1, h * 512 : (h + 1) * 512],
                    start=True,
                    stop=True,
                )
            rbr = small.tile([128, 512], mybir.dt.float32, tag="rbr", name=f"rbr_{u}")
            nc.vector.reciprocal_approx_fast(out=rbr[:], in_=pb[:])
            for h01 in range(2):
                nc.vector.tensor_mul(
                    out=waT_sb[64 * h01 : 64 * h01 + 64, pu, q0 : q0 + 512],
                    in0=pw_t[u][h01][0:64, :],
                    in1=rbr[64 * h01 : 64 * h01 + 64, :],
                )
            debt[0] += 700.0

        def emit_av(u, kc):
            q4u, pu = divmod(u, 4)
            if kc == 0:
                pw_t[u] = [
                    ps_wa.tile([65, 512], mybir.dt.float32, tag="wa", name=f"wa_{u}_{h}")
                    for h in range(2)
                ]
            for h01 in range(2):
                nc.tensor.matmul(
                    pw_t[u][h01][:],
                    lhsT=V_sb[:, kc, 2 * pu + h01, :],
                    rhs=E_t[u][:, kc, h01 * 512 : (h01 + 1) * 512],
                    start=(kc == 0),
                    stop=(kc == KC - 1),
                )

        for u in range(NU):
            q4, pair = divmod(u, 4)
            q0 = q4 * 512
            E_t[u] = epool.tile([128, KC, 1024], mybir.dt.bfloat16, tag="E", name=f"E_{u}")
            for ks in range(8):  # 2-kc slots: batch same-shape matmuls
                g = u * 16 + 2 * ks
                if u == NU - 1 and ks == 0:
                    flush(10 ** 9)  # drain all filler while still overlapped
                else:
                    flush(g + 1)
                psc = []
                for j in range(2):
                    kc = 2 * ks + j
                    p_t = ps_s.tile([128, 1024], mybir.dt.float32, tag="sc", name=f"sc_{u}_{kc}")
                    psc.append(p_t)
                    for h01 in range(2):
                        row = 64 * h01
                        nc.tensor.matmul(
                            p_t[:, h01 * 512 : (h01 + 1) * 512],
                            lhsT=qkT_sb[row : row + 64, 2 * pair + 1, kc * 128 : (kc + 1) * 128],
                            rhs=qkT_sb[row : row + 64, 2 * pair, q0 : q0 + 512],
                            start=True,
                            stop=True,
                        )
                for j in range(2):
                    nc.scalar.activation(
                        out=E_t[u][:, 2 * ks + j, :],
                        in_=psc[j][:],
                        func=mybir.ActivationFunctionType.Exp,
                        scale=0.125,
                    )
                if u > 0:
                    emit_av(u - 1, 2 * ks)
                    emit_av(u - 1, 2 * ks + 1)
                if u == NU - 1 and ks > 0:  # drain last unit with 1-slot lag
                    emit_av(u, 2 * (ks - 1))
                    emit_av(u, 2 * (ks - 1) + 1)
                debt[0] += (640.0 if u > 0 else 215.0) * 2 - 2 * SLOT_NS
                flush(g + 3)
                pull(g)
            if u > 0:
                emit_norm(u - 1)
                if (u - 1) % 4 == 3:
                    oq4 = (u - 1) // 4
                    for cc in range(4):
                        fill_dl = u * 16 + 12 * cc + 2
                        fq.append((fill_dl, 0, 0, 1900, (lambda c=oq4 * 4 + cc: emit_out_chain(c))))
                    fq = deque(sorted(fq))

        # drain: last AV slot, final norm, final out-proj
        emit_av(NU - 1, 14)
        emit_av(NU - 1, 15)
        emit_norm(NU - 1)
        while fq:
            fq.popleft()[4]()
        for cc in range(4):
            emit_out_chain(12 + cc)

    nc.compile()
    return nc


def _prep_in_maps(x, w_qkv, b_qkv, w_out):
    """Host-side shard + relayout. Core c -> (batch c//2, head-group c%2)."""
    wq = w_qkv[:, :D].reshape(D, H, HD)
    wk = w_qkv[:, D : 2 * D].reshape(D, H, HD)
    wv_ = w_qkv[:, 2 * D :].reshape(D, H, HD)
    bq = b_qkv[:D].reshape(H, HD)
    bk = b_qkv[D : 2 * D].reshape(H, HD)
    wo = w_out.reshape(H, HD, D)

    per_group = {}
    for g in range(G):
        h0 = g * LH
        # qk feature order: chunk 2p = q feats of heads (h0+2p, h0+2p+1)
        # (first head in cols 0-63), chunk 2p+1 = matching k feats.
        Wqk = np.empty((D, DC, 128), F32)
        Bqk = np.empty((DC, 128), F32)
        for p in range(LH // 2):
            ha, hb = h0 + 2 * p, h0 + 2 * p + 1
            Wqk[:, 2 * p, 0:64] = wq[:, ha]
            Wqk[:, 2 * p, 64:128] = wq[:, hb]
            Wqk[:, 2 * p + 1, 0:64] = wk[:, ha]
            Wqk[:, 2 * p + 1, 64:128] = wk[:, hb]
            Bqk[2 * p, 0:64] = bq[ha]
            Bqk[2 * p, 64:128] = bq[hb]
            Bqk[2 * p + 1, 0:64] = bk[ha]
            Bqk[2 * p + 1, 64:128] = bk[hb]
        wqk_arr = np.ascontiguousarray(
            Wqk.reshape(DC, 128, DC * 128).transpose(1, 0, 2)
        ).astype(BF16)
        bqk_arr = np.ascontiguousarray(Bqk.T)

        Wv = wv_[:, h0 : h0 + LH, :].reshape(D, LH * HD)
        wv_arr = np.ascontiguousarray(
            Wv.reshape(DC, 128, LH * HD).transpose(1, 0, 2)
        ).astype(BF16)

        Wo = wo[h0 : h0 + LH].reshape(LH * HD, D)
        wout_arr = np.ascontiguousarray(
            Wo.reshape(LH * HD // 128, 128, D).transpose(1, 0, 2)
        ).astype(BF16)
        per_group[g] = (wqk_arr, bqk_arr, wv_arr, wout_arr)

    in_maps = []
    for c in range(NCORES):
        b, g = divmod(c, G)
        wqk_arr, bqk_arr, wv_arr, wout_arr = per_group[g]
        xT_arr = np.ascontiguousarray(
            x[b].T.reshape(DC, 128, N).transpose(1, 0, 2)
        ).astype(BF16)
        in_maps.append(
            {
                "xT": xT_arr,
                "wqk": wqk_arr,
                "bqk": bqk_arr,
                "wv": wv_arr,
                "wout": wout_arr,
            }
        )
    return in_maps


def _ensure_ntff_hook():
    """Register the axon NTFF profile hook if the image's antenv lacks it."""
    try:
        from antenv.axon_hooks import get_axon_ntff_profile_hook  # noqa: F401

        return
    except ImportError:
        pass

    import contextlib
    import ctypes
    import types

    so_path = "/opt/axon/libaxon_pjrt.so"
    lib = ctypes.CDLL(so_path)
    if not hasattr(lib, "axon_start_nrt_profile"):
        return
    lib.axon_start_nrt_profile.argtypes = [ctypes.POINTER(ctypes.c_int64), ctypes.c_size_t]
    lib.axon_start_nrt_profile.restype = ctypes.c_int64
    lib.axon_stop_nrt_profile.argtypes = [ctypes.c_char_p]
    lib.axon_stop_nrt_profile.restype = ctypes.c_int64

    @contextlib.contextmanager
    def _hook(output_dir, device_ids):
        import jax

        jax.devices()
        if device_ids:
            ids = (ctypes.c_int64 * len(device_ids))(*device_ids)
            rc = lib.axon_start_nrt_profile(ids, len(device_ids))
        else:
            rc = lib.axon_start_nrt_profile(None, 0)
        if rc != 0:
            raise RuntimeError(f"axon_start_nrt_profile rc={rc}")
        try:
            yield
        finally:
            n = lib.axon_stop_nrt_profile(str(output_dir).encode())
            print(f"ntff profile: {n} file(s) written to {output_dir}", file=sys.stderr)

    mod = types.ModuleType("antenv.axon_hooks")
    mod.get_axon_ntff_profile_hook = lambda: _hook
    sys.modules["antenv.axon_hooks"] = mod

    from concourse import bass_utils as _bu

    _bu.upload_artifacts = lambda tmpdir: tmpdir


def kernel(x, w_qkv, b_qkv, w_out, b_out):
    x = np.asarray(x, dtype=F32)
    w_qkv = np.asarray(w_qkv, dtype=F32)
    b_qkv = np.asarray(b_qkv, dtype=F32)
    w_out = np.asarray(w_out, dtype=F32)
    b_out = np.asarray(b_out, dtype=F32)

    if "nc" not in _CACHE:
        _CACHE["nc"] = _build_nc()
    nc = _CACHE["nc"]

    in_maps = _prep_in_maps(x, w_qkv, b_qkv, w_out)
    trace = bool(int(os.environ.get("BASSMHA_TRACE", "0")))
    kwargs = {}
    if trace:
        _ensure_ntff_hook()
        tdir = os.environ.get("BASSMHA_TRACE_DIR")
        if tdir:
            os.makedirs(tdir, exist_ok=True)
            kwargs["tmpdir"] = tdir
    res = run_bass_kernel_spmd(nc, in_maps, list(range(NCORES)), trace=trace, **kwargs)
    _CACHE["last_results"] = res

    # v-bias folded out of the device kernel: attention weights sum to 1,
    # so the bias contributes exactly bv @ w_out per token.
    bias_row = b_out + b_qkv[2 * D :].astype(F32) @ w_out
    out = np.empty((B, N, D), F32)
    for b in range(B):
        out[b] = res.results[2 * b]["out"].astype(F32)
        out[b] += res.results[2 * b + 1]["out"].astype(F32)
        out[b] += bias_row
    return out


# revision 12
# speedup vs baseline: 1.0012x; 1.0012x over previous
"""Multi-head self-attention on 8 Trainium2 NeuronCores.

Problem: x[4, 2048, 1024], 16 heads x 64 dims, fused qkv + attention + out-proj.

Sharding (hybrid, per the tensor-parallel hint): core c handles batch b = c//2
and head-group g = c%2 (8 of the 16 heads). Each core computes a partial
out-projection over its 8 heads; the host sums the two group partials per
batch and adds b_out (+ the folded v-bias term, see below).

The kernel is ACT(exp)-bound: 256 exp tiles of [128, 1024] at ~1.15us each
(~294us of scalar-engine work). Everything else is scheduled around keeping
ACT saturated from ~5us onward:
  - scores computed transposed (S^T[k, q]) per 128-row k-chunk; two heads of
    a pair row-packed on the PE (K=64 each) into one [128, 1024] PSUM tile
    that a single exp covers; exp reads PSUM, writes bf16 E to SBUF.
  - softmax denominator comes free as an all-ones column appended to V in
    the AV matmul (row 64 of the PSUM result).
  - normalization without ACT: PE rank-1 broadcast of the bf16 denominators,
    one DVE reciprocal_approx_fast on the [128, 512] broadcast (reads PSUM),
    then two DVE muls -> normalized waT.  No Ln/Exp table work.
  - v-bias folded out on the host: softmax weights sum to 1 exactly (the
    denominator IS the ones-column sum), so the bias contributes bv @ w_out,
    added to b_out host-side.  q/k biases stay in the projection.
  - emission is software-pipelined: the scores->exp->AV stream is the
    backbone (unit u's AV emitted during unit u+1); qkproj / vproj /
    out-proj chains are deadline-scheduled filler inside the stream, and
    input DMAs are sliced so the first scores matmul can start ~4us in.
"""

import os
import sys
from contextlib import ExitStack

import numpy as np

for _p in ("/opt/trn_rl_repo",):
    if _p not in sys.path and os.path.isdir(_p):
        sys.path.insert(0, _p)

import ml_dtypes

import concourse.bass as bass
import concourse.tile as tile
from concourse import bacc, mybir
from concourse.bass_utils import run_bass_kernel_spmd

BF16 = ml_dtypes.bfloat16
F32 = np.float32

D = 1024
H = 16
HD = 64
B = 4
N = 2048
NCORES = 8
G = 2  # head groups (tensor-parallel axis)
LH = H // G  # local heads per core
DC = D // 128  # 8 contraction chunks
KC = N // 128  # 16 k-token chunks
QT = N // 512  # 4 q tiles
TOK = N // 128  # 16 token chunks
NU = 16  # pipeline units: u = q4*4 + pair

_CACHE = {}


def _pin_act_tables():
    """Pin the act-table chooser so exp resolves to one stable set (no
    mid-kernel table reloads)."""
    if _CACHE.get("act_pinned"):
        return
    from concourse import bacc as _bacc
    from concourse import hw_specs as _hw

    orig = _hw.get_activation_tables

    def patched(arch):
        t = dict(orig(arch))
        keep = "natural_log_exp_and_others"
        if keep in t:
            pinned = t[keep]
            t = {n: (s if n == keep else (s - pinned)) for n, s in t.items()}
        return t

    _hw.get_activation_tables = patched
    _bacc.get_activation_tables = patched
    _CACHE["act_pinned"] = True


def _build_nc():
    _pin_act_tables()
    nc = bacc.Bacc(None, target_bir_lowering=False)

    xT = nc.declare_dram_parameter("xT", [128, DC, N], mybir.dt.bfloat16, isOutput=False)
    wqk = nc.declare_dram_parameter("wqk", [128, DC, 2 * LH * HD], mybir.dt.bfloat16, isOutput=False)
    bqk = nc.declare_dram_parameter("bqk", [128, DC], mybir.dt.float32, isOutput=False)
    wv = nc.declare_dram_parameter("wv", [128, DC, LH * HD], mybir.dt.bfloat16, isOutput=False)
    wout = nc.declare_dram_parameter("wout", [128, LH * HD // 128, D], mybir.dt.bfloat16, isOutput=False)
    out = nc.declare_dram_parameter("out", [N, D], mybir.dt.bfloat16, isOutput=True)

    with tile.TileContext(nc) as tc, ExitStack() as ctx:
        const = ctx.enter_context(tc.tile_pool(name="const", bufs=1))
        xpool = ctx.enter_context(tc.tile_pool(name="xpool", bufs=1))
        epool = ctx.enter_context(tc.tile_pool(name="epool", bufs=2))
        work = ctx.enter_context(tc.tile_pool(name="work", bufs=1))
        outp = ctx.enter_context(tc.tile_pool(name="outp", bufs=2))
        small = ctx.enter_context(tc.tile_pool(name="small", bufs=2))
        ps_s = ctx.enter_context(tc.tile_pool(name="ps_s", bufs=2, space="PSUM"))
        ps_wa = ctx.enter_context(tc.tile_pool(name="ps_wa", bufs=2, space="PSUM"))
        ps_m = ctx.enter_context(tc.tile_pool(name="ps_m", bufs=2, space="PSUM"))

        wqk_sb = const.tile([128, DC, 2 * LH * HD], mybir.dt.bfloat16)
        bqk_sb = const.tile([128, DC], mybir.dt.float32)
        wv_sb = const.tile([128, DC, LH * HD], mybir.dt.bfloat16)
        wout_sb = const.tile([128, LH * HD // 128, D], mybir.dt.bfloat16)
        ones_bf = const.tile([1, 128], mybir.dt.bfloat16)
        xT_sb = xpool.tile([128, DC, N], mybir.dt.bfloat16, tag="xT")
        qkT_sb = work.tile([128, DC, N], mybir.dt.bfloat16, tag="qkT")
        V_sb = work.tile([128, KC, LH, HD + 1], mybir.dt.bfloat16, tag="V")
        waT_sb = work.tile([128, LH * HD // 128, N], mybir.dt.bfloat16, tag="waT")

        # --- DMA: sliced so the first qk chains can start early ------------
        def dma_x(tt, chunked=False):
            # tt0 per contraction-chunk (warm-up pacers key off each chunk);
            # later slices as one descriptor to cut Sync issue serialization
            if chunked:
                for kc in range(DC):
                    nc.sync.dma_start(
                        out=xT_sb[:, kc, tt * 512 : (tt + 1) * 512],
                        in_=xT[:, kc, tt * 512 : (tt + 1) * 512],
                    )
            else:
                nc.sync.dma_start(
                    out=xT_sb[:, :, tt * 512 : (tt + 1) * 512],
                    in_=xT[:, :, tt * 512 : (tt + 1) * 512],
                )

        def dma_wqk(p):
            nc.sync.dma_start(
                out=wqk_sb[:, :, p * 256 : (p + 1) * 256],
                in_=wqk[:, :, p * 256 : (p + 1) * 256],
            )

        dma_wqk(0)
        nc.sync.dma_start(out=bqk_sb[:], in_=bqk[:])
        dma_x(0, chunked=True)
        nc.sync.dma_start(out=wv_sb[:], in_=wv[:])
        for tt in range(1, QT):
            dma_x(tt)
        for p in range(1, LH // 2):
            dma_wqk(p)
        nc.sync.dma_start(out=wout_sb[:], in_=wout[:])

        # earliest slot (in kc units, ~1.15us each) each DMA'd tensor is
        # usable; gates the budget puller so a DMA-blocked chain never
        # head-of-line-stalls the PE queue.
        arr_x = [0, 4, 6, 8]
        arr_wqk = [0, 10, 11, 12]
        ARR_WV = 2

        nc.vector.memset(ones_bf[:], 1.0)
        # only the ones column (index HD) of V needs initializing -- it feeds
        # the free softmax denominator; v-proj fills [0:HD].
        nc.vector.memset(V_sb[:, :, :, HD : HD + 1], 1.0)

        # HAM warm-up: a burst of junk matmuls unthrottles the PE clock
        # (~3.4us busy window), then one pacer MM per arriving x-chunk keeps
        # it warm across the DMA lead-in. Results go to a scratch PSUM tile
        # in the scores ring; nothing reads them.
        warm_ps = ps_s.tile([128, 1024], mybir.dt.float32, tag="sc", name="warm_ps")
        for i in range(110):
            nc.tensor.matmul(
                warm_ps[0:128, 0:64],
                lhsT=ones_bf[0:1, 0:128],
                rhs=ones_bf[0:1, 0:64],
                start=True,
                stop=True,
            )

        def emit_pacer(n):
            # junk matmuls that fill PE idle pockets so HAM stays at K=8/8
            for i in range(n):
                nc.tensor.matmul(
                    warm_ps[0:128, 0:128],
                    lhsT=ones_bf[0:1, 0:128],
                    rhs=ones_bf[0:1, 0:128],
                    start=True,
                    stop=True,
                )
        for kc in range(DC):
            nc.tensor.matmul(
                warm_ps[0:64, 0:64],
                lhsT=xT_sb[:, kc, 0:64],
                rhs=xT_sb[:, kc, 0:64],
                start=True,
                stop=True,
            )

        # --- filler work items (deadline-scheduled PE chains) --------------
        def emit_qk_chain(m, tt):
            pq = ps_m.tile([128, 512], mybir.dt.float32, tag="misc", name=f"pq_{m}_{tt}")
            for kc in range(DC):
                nc.tensor.matmul(
                    pq[:],
                    lhsT=wqk_sb[:, kc, m * 128 : (m + 1) * 128],
                    rhs=xT_sb[:, kc, tt * 512 : (tt + 1) * 512],
                    start=(kc == 0),
                    stop=(kc == DC - 1),
                )
            nc.vector.tensor_scalar_add(
                out=qkT_sb[:, m, tt * 512 : (tt + 1) * 512],
                in0=pq[:],
                scalar1=bqk_sb[:, m : m + 1],
            )

        def emit_v_chain(c):
            pv = ps_m.tile([128, 512], mybir.dt.float32, tag="misc", name=f"pv_{c}")
            for kc in range(DC):
                nc.tensor.matmul(
                    pv[:],
                    lhsT=xT_sb[:, kc, c * 128 : (c + 1) * 128],
                    rhs=wv_sb[:, kc, :],
                    start=(kc == 0),
                    stop=(kc == DC - 1),
                )
            nc.vector.tensor_copy(
                out=V_sb[:, c, :, 0:HD],
                in_=pv[:].rearrange("p (l d) -> p l d", l=LH),
            )

        def emit_out_chain(c):
            # k4-outer with both half-D accumulators open: 8 same-shape MMs
            # back-to-back (each lhsT used for both halves)
            po = [
                ps_m.tile([128, 512], mybir.dt.float32, tag="misc", name=f"po_{c}_{h}")
                for h in range(2)
            ]
            for k4 in range(LH * HD // 128):
                for half in range(2):
                    nc.tensor.matmul(
                        po[half][:],
                        lhsT=waT_sb[:, k4, c * 128 : (c + 1) * 128],
                        rhs=wout_sb[:, k4, half * 512 : (half + 1) * 512],
                        start=(k4 == 0),
                        stop=(k4 == LH * HD // 128 - 1),
                    )
            for half in range(2):
                o_sb = outp.tile([128, 512], mybir.dt.bfloat16, tag="osb", name=f"o_{c}_{half}")
                nc.vector.tensor_copy(out=o_sb[:], in_=po[half][:])
                nc.sync.dma_start(
                    out=out[c * 128 : (c + 1) * 128, half * 512 : (half + 1) * 512],
                    in_=o_sb[:],
                )

        # items: (deadline_g, seq, earliest_g, cost_ns, fn); deadline/earliest
        # in kc-slot units g = u*16 + kc
        fill = []
        seq = 0
        for p in range(LH // 2):
            for tt in range(QT):
                e = max(arr_x[tt], arr_wqk[p])
                fill.append((p * 16 + 4 * tt, seq, e, 1750,
                             (lambda m=2 * p + 1, t=tt: emit_qk_chain(m, t))))
                seq += 1
                fill.append(((4 * tt + p) * 16, seq, e, 1750,
                             (lambda m=2 * p, t=tt: emit_qk_chain(m, t))))
                seq += 1
        for c in range(TOK):
            fill.append((max(6 + c, arr_x[c // 4] + 1), seq,
                         max(arr_x[c // 4], ARR_WV), 1750,
                         (lambda cc=c: emit_v_chain(cc))))
            seq += 1
        fill.sort()
        from collections import deque

        fq = deque(fill)

        SLOT_NS = 1150.0  # ACT pace per kc
        debt = [0.0]

        def flush(g):
            while fq and fq[0][0] <= g:
                _, _, _, cns, fn = fq.popleft()
                fn()
                debt[0] += cns

        def pull(g):
            # pop DMA-ready items while the PE has slack
            while debt[0] < 300.0 and fq:
                if fq[0][2] > g:
                    break
                _, _, _, cns, fn = fq.popleft()
                fn()
                debt[0] += cns
            if debt[0] < -20000.0:
                debt[0] = -20000.0

        # --- the pipelined attention stream --------------------------------
        E_t = [None] * NU
        pw_t = [None] * NU

        def emit_norm(u):
            """Normalize unit u's AV output into waT (no ACT work): gather the
            two denominator rows, one rank-1 broadcast matmul into a scores-
            ring PSUM tile, one DVE reciprocal, two DVE muls."""
            q4u, pu = divmod(u, 4)
            q0 = q4u * 512
            den2 = small.tile([1, 1024], mybir.dt.bfloat16, tag="den", name=f"den_{u}")
            nc.vector.tensor_copy(out=den2[0:1, 0:512], in_=pw_t[u][0][64:65, :])
            nc.vector.tensor_copy(out=den2[0:1, 512:1024], in_=pw_t[u][1][64:65, :])
            pb = ps_m.tile([128, 512], mybir.dt.float32, tag="misc", name=f"pb_{u}")
            for h in range(2):
                nc.tensor.matmul(
                    pb[64 * h : 64 * h + 64, :],
                    lhsT=ones_bf[0:1, 0:64],
                    rhs=den2[0:1, h * 512 : (h + 1) * 512],
                    start=True,
                    stop=True,
                )
            rbr = small.tile([128, 512], mybir.dt.float32, tag="rbr", name=f"rbr_{u}")
            nc.vector.reciprocal_approx_fast(out=rbr[:], in_=pb[:])
            for h01 in range(2):
                nc.vector.tensor_mul(
                    out=waT_sb[64 * h01 : 64 * h01 + 64, pu, q0 : q0 + 512],
                    in0=pw_t[u][h01][0:64, :],
                    in1=rbr[64 * h01 : 64 * h01 + 64, :],
                )
            debt[0] += 700.0

        def emit_av(u, kc):
            q4u, pu = divmod(u, 4)
            if kc == 0:
                pw_t[u] = [
                    ps_wa.tile([65, 512], mybir.dt.float32, tag="wa", name=f"wa_{u}_{h}")
                    for h in range(2)
                ]
            for h01 in range(2):
                nc.tensor.matmul(
                    pw_t[u][h01][:],
                    lhsT=V_sb[:, kc, 2 * pu + h01, :],
                    rhs=E_t[u][:, kc, h01 * 512 : (h01 + 1) * 512],
                    start=(kc == 0),
                    stop=(kc == KC - 1),
                )

        for u in range(NU):
            q4, pair = divmod(u, 4)
            q0 = q4 * 512
            E_t[u] = epool.tile([128, KC, 1024], mybir.dt.bfloat16, tag="E", name=f"E_{u}")
            for ks in range(8):  # 2-kc slots: batch same-shape matmuls
                g = u * 16 + 2 * ks
                if u == NU - 1 and ks == 0:
                    flush(10 ** 9)  # drain all filler while still overlapped
                else:
                    flush(g + 1)
                psc = []
                for j in range(2):
                    kc = 2 * ks + j
                    p_t = ps_s.tile([128, 1024], mybir.dt.float32, tag="sc", name=f"sc_{u}_{kc}")
                    psc.append(p_t)
                    for h01 in range(2):
                        row = 64 * h01
                        nc.tensor.matmul(
                            p_t[:, h01 * 512 : (h01 + 1) * 512],
                            lhsT=qkT_sb[row : row + 64, 2 * pair + 1, kc * 128 : (kc + 1) * 128],
                            rhs=qkT_sb[row : row + 64, 2 * pair, q0 : q0 + 512],
                            start=True,
                            stop=True,
                        )
                for j in range(2):
                    nc.scalar.activation(
                        out=E_t[u][:, 2 * ks + j, :],
                        in_=psc[j][:],
                        func=mybir.ActivationFunctionType.Exp,
                        scale=0.125,
                    )
                if u > 0:
                    emit_av(u - 1, 2 * ks)
                    emit_av(u - 1, 2 * ks + 1)
                if u == NU - 1 and ks > 0:  # drain last unit with 1-slot lag
                    emit_av(u, 2 * (ks - 1))
                    emit_av(u, 2 * (ks - 1) + 1)
                debt[0] += (640.0 if u > 0 else 215.0) * 2 - 2 * SLOT_NS
                flush(g + 3)
                pull(g)
            if u > 0:
                emit_norm(u - 1)
                if (u - 1) % 4 == 3:
                    oq4 = (u - 1) // 4
                    for cc in range(4):
                        fill_dl = u * 16 + 12 * cc + 2
                        fq.append((fill_dl, 0, 0, 1900, (lambda c=oq4 * 4 + cc: emit_out_chain(c))))
                    fq = deque(sorted(fq))

        # drain: last AV slot, final norm, final out-proj; pacer matmuls
        # keep the PE clock warm through the dependency pockets
        emit_av(NU - 1, 14)
        emit_av(NU - 1, 15)
        emit_pacer(4)
        emit_norm(NU - 1)
        while fq:
            fq.popleft()[4]()
        for cc in range(4):
            emit_pacer(3)
            emit_out_chain(12 + cc)

    nc.compile()
    return nc


def _prep_in_maps(x, w_qkv, b_qkv, w_out):
    """Host-side shard + relayout. Core c -> (batch c//2, head-group c%2)."""
    wq = w_qkv[:, :D].reshape(D, H, HD)
    wk = w_qkv[:, D : 2 * D].reshape(D, H, HD)
    wv_ = w_qkv[:, 2 * D :].reshape(D, H, HD)
    bq = b_qkv[:D].reshape(H, HD)
    bk = b_qkv[D : 2 * D].reshape(H, HD)
    wo = w_out.reshape(H, HD, D)

    per_group = {}
    for g in range(G):
        h0 = g * LH
        # qk feature order: chunk 2p = q feats of heads (h0+2p, h0+2p+1)
        # (first head in cols 0-63), chunk 2p+1 = matching k feats.
        Wqk = np.empty((D, DC, 128), F32)
        Bqk = np.empty((DC, 128), F32)
        for p in range(LH // 2):
            ha, hb = h0 + 2 * p, h0 + 2 * p + 1
            Wqk[:, 2 * p, 0:64] = wq[:, ha]
            Wqk[:, 2 * p, 64:128] = wq[:, hb]
            Wqk[:, 2 * p + 1, 0:64] = wk[:, ha]
            Wqk[:, 2 * p + 1, 64:128] = wk[:, hb]
            Bqk[2 * p, 0:64] = bq[ha]
            Bqk[2 * p, 64:128] = bq[hb]
            Bqk[2 * p + 1, 0:64] = bk[ha]
            Bqk[2 * p + 1, 64:128] = bk[hb]
        wqk_arr = np.ascontiguousarray(
            Wqk.reshape(DC, 128, DC * 128).transpose(1, 0, 2)
        ).astype(BF16)
        bqk_arr = np.ascontiguousarray(Bqk.T)

        Wv = wv_[:, h0 : h0 + LH, :].reshape(D, LH * HD)
        wv_arr = np.ascontiguousarray(
            Wv.reshape(DC, 128, LH * HD).transpose(1, 0, 2)
        ).astype(BF16)

        Wo = wo[h0 : h0 + LH].reshape(LH * HD, D)
        wout_arr = np.ascontiguousarray(
            Wo.reshape(LH * HD // 128, 128, D).transpose(1, 0, 2)
        ).astype(BF16)
        per_group[g] = (wqk_arr, bqk_arr, wv_arr, wout_arr)

    in_maps = []
    for c in range(NCORES):
        b, g = divmod(c, G)
        wqk_arr, bqk_arr, wv_arr, wout_arr = per_group[g]
        xT_arr = np.ascontiguousarray(
            x[b].T.reshape(DC, 128, N).transpose(1, 0, 2)
        ).astype(BF16)
        in_maps.append(
            {
                "xT": xT_arr,
                "wqk": wqk_arr,
                "bqk": bqk_arr,
                "wv": wv_arr,
                "wout": wout_arr,
            }
        )
    return in_maps


def _ensure_ntff_hook():
    """Register the axon NTFF profile hook if the image's antenv lacks it."""
    try:
        from antenv.axon_hooks import get_axon_ntff_profile_hook  # noqa: F401

        return
    except ImportError:
        pass

    import contextlib
    import ctypes
    import types

    so_path = "/opt/axon/libaxon_pjrt.so"
    lib = ctypes.CDLL(so_path)
    if not hasattr(lib, "axon_start_nrt_profile"):
        return
    lib.axon_start_nrt_profile.argtypes = [ctypes.POINTER(ctypes.c_int64), ctypes.c_size_t]
    lib.axon_start_nrt_profile.restype = ctypes.c_int64
    lib.axon_stop_nrt_profile.argtypes = [ctypes.c_char_p]
    lib.axon_stop_nrt_profile.restype = ctypes.c_int64

    @contextlib.contextmanager
    def _hook(output_dir, device_ids):
        import jax

        jax.devices()
        if device_ids:
            ids = (ctypes.c_int64 * len(device_ids))(*device_ids)
            rc = lib.axon_start_nrt_profile(ids, len(device_ids))
        else:
            rc = lib.axon_start_nrt_profile(None, 0)
        if rc != 0:
            raise RuntimeError(f"axon_start_nrt_profile rc={rc}")
        try:
            yield
        finally:
            n = lib.axon_stop_nrt_profile(str(output_dir).encode())
            print(f"ntff profile: {n} file(s) written to {output_dir}", file=sys.stderr)

    mod = types.ModuleType("antenv.axon_hooks")
    mod.get_axon_ntff_profile_hook = lambda: _hook
    sys.modules["antenv.axon_hooks"] = mod

    from concourse import bass_utils as _bu

    _bu.upload_artifacts = lambda tmpdir: tmpdir


def kernel(x, w_qkv, b_qkv, w_out, b_out):
    x = np.asarray(x, dtype=F32)
    w_qkv = np.asarray(w_qkv, dtype=F32)
    b_qkv = np.asarray(b_qkv, dtype=F32)
    w_out = np.asarray(w_out, dtype=F32)
    b_out = np.asarray(b_out, dtype=F32)

    if "nc" not in _CACHE:
        _CACHE["nc"] = _build_nc()
    nc = _CACHE["nc"]

    in_maps = _prep_in_maps(x, w_qkv, b_qkv, w_out)
    trace = bool(int(os.environ.get("BASSMHA_TRACE", "0")))
    kwargs = {}
    if trace:
        _ensure_ntff_hook()
        tdir = os.environ.get("BASSMHA_TRACE_DIR")
        if tdir:
            os.makedirs(tdir, exist_ok=True)
            kwargs["tmpdir"] = tdir
    res = run_bass_kernel_spmd(nc, in_maps, list(range(NCORES)), trace=trace, **kwargs)
    _CACHE["last_results"] = res

    # v-bias folded out of the device kernel: attention weights sum to 1,
    # so the bias contributes exactly bv @ w_out per token.
    bias_row = b_out + b_qkv[2 * D :].astype(F32) @ w_out
    out = np.empty((B, N, D), F32)
    for b in range(B):
        out[b] = res.results[2 * b]["out"].astype(F32)
        out[b] += res.results[2 * b + 1]["out"].astype(F32)
        out[b] += bias_row
    return out


# revision 13
# speedup vs baseline: 1.0019x; 1.0007x over previous
"""Multi-head self-attention on 8 Trainium2 NeuronCores.

Problem: x[4, 2048, 1024], 16 heads x 64 dims, fused qkv + attention + out-proj.

Sharding (hybrid, per the tensor-parallel hint): core c handles batch b = c//2
and head-group g = c%2 (8 of the 16 heads). Each core computes a partial
out-projection over its 8 heads; the host sums the two group partials per
batch and adds b_out (+ the folded v-bias term, see below).

The kernel is ACT(exp)-bound: 256 exp tiles of [128, 1024] at ~1.15us each
(~294us of scalar-engine work). Everything else is scheduled around keeping
ACT saturated from ~5us onward:
  - scores computed transposed (S^T[k, q]) per 128-row k-chunk; two heads of
    a pair row-packed on the PE (K=64 each) into one [128, 1024] PSUM tile
    that a single exp covers; exp reads PSUM, writes bf16 E to SBUF.
  - softmax denominator comes free as an all-ones column appended to V in
    the AV matmul (row 64 of the PSUM result).
  - normalization without ACT: PE rank-1 broadcast of the bf16 denominators,
    one DVE reciprocal_approx_fast on the [128, 512] broadcast (reads PSUM),
    then two DVE muls -> normalized waT.  No Ln/Exp table work.
  - v-bias folded out on the host: softmax weights sum to 1 exactly (the
    denominator IS the ones-column sum), so the bias contributes bv @ w_out,
    added to b_out host-side.  q/k biases stay in the projection.
  - emission is software-pipelined: the scores->exp->AV stream is the
    backbone (unit u's AV emitted during unit u+1); qkproj / vproj /
    out-proj chains are deadline-scheduled filler inside the stream, and
    input DMAs are sliced so the first scores matmul can start ~4us in.
"""

import os
import sys
from contextlib import ExitStack

import numpy as np

for _p in ("/opt/trn_rl_repo",):
    if _p not in sys.path and os.path.isdir(_p):
        sys.path.insert(0, _p)

import ml_dtypes

import concourse.bass as bass
import concourse.tile as tile
from concourse import bacc, mybir
from concourse.bass_utils import run_bass_kernel_spmd

BF16 = ml_dtypes.bfloat16
F32 = np.float32

D = 1024
H = 16
HD = 64
B = 4
N = 2048
NCORES = 8
G = 2  # head groups (tensor-parallel axis)
LH = H // G  # local heads per core
DC = D // 128  # 8 contraction chunks
KC = N // 128  # 16 k-token chunks
QT = N // 512  # 4 q tiles
TOK = N // 128  # 16 token chunks
NU = 16  # pipeline units: u = q4*4 + pair

_CACHE = {}


def _pin_act_tables():
    """Pin the act-table chooser so exp resolves to one stable set (no
    mid-kernel table reloads)."""
    if _CACHE.get("act_pinned"):
        return
    from concourse import bacc as _bacc
    from concourse import hw_specs as _hw

    orig = _hw.get_activation_tables

    def patched(arch):
        t = dict(orig(arch))
        keep = "natural_log_exp_and_others"
        if keep in t:
            pinned = t[keep]
            t = {n: (s if n == keep else (s - pinned)) for n, s in t.items()}
        return t

    _hw.get_activation_tables = patched
    _bacc.get_activation_tables = patched
    _CACHE["act_pinned"] = True


def _build_nc():
    _pin_act_tables()
    nc = bacc.Bacc(None, target_bir_lowering=False)

    xT = nc.declare_dram_parameter("xT", [128, DC, N], mybir.dt.bfloat16, isOutput=False)
    wqk = nc.declare_dram_parameter("wqk", [128, DC, 2 * LH * HD], mybir.dt.bfloat16, isOutput=False)
    bqk = nc.declare_dram_parameter("bqk", [128, DC], mybir.dt.float32, isOutput=False)
    wv = nc.declare_dram_parameter("wv", [128, DC, LH * HD], mybir.dt.bfloat16, isOutput=False)
    wout = nc.declare_dram_parameter("wout", [128, LH * HD // 128, D], mybir.dt.bfloat16, isOutput=False)
    out = nc.declare_dram_parameter("out", [N, D], mybir.dt.bfloat16, isOutput=True)

    with tile.TileContext(nc) as tc, ExitStack() as ctx:
        const = ctx.enter_context(tc.tile_pool(name="const", bufs=1))
        xpool = ctx.enter_context(tc.tile_pool(name="xpool", bufs=1))
        epool = ctx.enter_context(tc.tile_pool(name="epool", bufs=2))
        work = ctx.enter_context(tc.tile_pool(name="work", bufs=1))
        outp = ctx.enter_context(tc.tile_pool(name="outp", bufs=2))
        small = ctx.enter_context(tc.tile_pool(name="small", bufs=2))
        ps_s = ctx.enter_context(tc.tile_pool(name="ps_s", bufs=2, space="PSUM"))
        ps_wa = ctx.enter_context(tc.tile_pool(name="ps_wa", bufs=2, space="PSUM"))
        ps_m = ctx.enter_context(tc.tile_pool(name="ps_m", bufs=2, space="PSUM"))

        wqk_sb = const.tile([128, DC, 2 * LH * HD], mybir.dt.bfloat16)
        bqk_sb = const.tile([128, DC], mybir.dt.float32)
        wv_sb = const.tile([128, DC, LH * HD], mybir.dt.bfloat16)
        wout_sb = const.tile([128, LH * HD // 128, D], mybir.dt.bfloat16)
        ones_bf = const.tile([1, 128], mybir.dt.bfloat16)
        xT_sb = xpool.tile([128, DC, N], mybir.dt.bfloat16, tag="xT")
        qkT_sb = work.tile([128, DC, N], mybir.dt.bfloat16, tag="qkT")
        V_sb = work.tile([128, KC, LH, HD + 1], mybir.dt.bfloat16, tag="V")
        waT_sb = work.tile([128, LH * HD // 128, N], mybir.dt.bfloat16, tag="waT")

        # --- DMA: sliced so the first qk chains can start early ------------
        def dma_x(tt, chunked=False):
            # tt0 per contraction-chunk (warm-up pacers key off each chunk);
            # later slices as one descriptor to cut Sync issue serialization
            if chunked:
                for kc in range(DC):
                    nc.sync.dma_start(
                        out=xT_sb[:, kc, tt * 512 : (tt + 1) * 512],
                        in_=xT[:, kc, tt * 512 : (tt + 1) * 512],
                    )
            else:
                nc.sync.dma_start(
                    out=xT_sb[:, :, tt * 512 : (tt + 1) * 512],
                    in_=xT[:, :, tt * 512 : (tt + 1) * 512],
                )

        def dma_wqk(p):
            nc.sync.dma_start(
                out=wqk_sb[:, :, p * 256 : (p + 1) * 256],
                in_=wqk[:, :, p * 256 : (p + 1) * 256],
            )

        dma_wqk(0)
        nc.sync.dma_start(out=bqk_sb[:], in_=bqk[:])
        dma_x(0, chunked=True)
        nc.sync.dma_start(out=wv_sb[:], in_=wv[:])
        for tt in range(1, QT):
            dma_x(tt)
        for p in range(1, LH // 2):
            dma_wqk(p)
        nc.sync.dma_start(out=wout_sb[:], in_=wout[:])

        # earliest slot (in kc units, ~1.15us each) each DMA'd tensor is
        # usable; gates the budget puller so a DMA-blocked chain never
        # head-of-line-stalls the PE queue.
        arr_x = [0, 4, 6, 8]
        arr_wqk = [0, 10, 11, 12]
        ARR_WV = 2

        nc.vector.memset(ones_bf[:], 1.0)
        # only the ones column (index HD) of V needs initializing -- it feeds
        # the free softmax denominator; v-proj fills [0:HD].
        nc.vector.memset(V_sb[:, :, :, HD : HD + 1], 1.0)

        # HAM warm-up: a burst of junk matmuls unthrottles the PE clock
        # (~3.4us busy window), then one pacer MM per arriving x-chunk keeps
        # it warm across the DMA lead-in. Results go to a scratch PSUM tile
        # in the scores ring; nothing reads them.
        warm_ps = ps_s.tile([128, 1024], mybir.dt.float32, tag="sc", name="warm_ps")
        for i in range(110):
            nc.tensor.matmul(
                warm_ps[0:128, 0:64],
                lhsT=ones_bf[0:1, 0:128],
                rhs=ones_bf[0:1, 0:64],
                start=True,
                stop=True,
            )

        def emit_pacer(n):
            # junk matmuls that fill PE idle pockets so HAM stays at K=8/8
            for i in range(n):
                nc.tensor.matmul(
                    warm_ps[0:128, 0:128],
                    lhsT=ones_bf[0:1, 0:128],
                    rhs=ones_bf[0:1, 0:128],
                    start=True,
                    stop=True,
                )
        for kc in range(DC):
            nc.tensor.matmul(
                warm_ps[0:64, 0:64],
                lhsT=xT_sb[:, kc, 0:64],
                rhs=xT_sb[:, kc, 0:64],
                start=True,
                stop=True,
            )

        # --- filler work items (deadline-scheduled PE chains) --------------
        def emit_qk_chain(m, tt):
            pq = ps_m.tile([128, 512], mybir.dt.float32, tag="misc", name=f"pq_{m}_{tt}")
            for kc in range(DC):
                nc.tensor.matmul(
                    pq[:],
                    lhsT=wqk_sb[:, kc, m * 128 : (m + 1) * 128],
                    rhs=xT_sb[:, kc, tt * 512 : (tt + 1) * 512],
                    start=(kc == 0),
                    stop=(kc == DC - 1),
                )
            nc.vector.tensor_scalar_add(
                out=qkT_sb[:, m, tt * 512 : (tt + 1) * 512],
                in0=pq[:],
                scalar1=bqk_sb[:, m : m + 1],
            )

        def emit_v_chain(c):
            pv = ps_m.tile([128, 512], mybir.dt.float32, tag="misc", name=f"pv_{c}")
            for kc in range(DC):
                nc.tensor.matmul(
                    pv[:],
                    lhsT=xT_sb[:, kc, c * 128 : (c + 1) * 128],
                    rhs=wv_sb[:, kc, :],
                    start=(kc == 0),
                    stop=(kc == DC - 1),
                )
            nc.vector.tensor_copy(
                out=V_sb[:, c, :, 0:HD],
                in_=pv[:].rearrange("p (l d) -> p l d", l=LH),
            )

        def emit_out_chain(c):
            # k4-outer with both half-D accumulators open: 8 same-shape MMs
            # back-to-back (each lhsT used for both halves)
            po = [
                ps_m.tile([128, 512], mybir.dt.float32, tag="misc", name=f"po_{c}_{h}")
                for h in range(2)
            ]
            for k4 in range(LH * HD // 128):
                for half in range(2):
                    nc.tensor.matmul(
                        po[half][:],
                        lhsT=waT_sb[:, k4, c * 128 : (c + 1) * 128],
                        rhs=wout_sb[:, k4, half * 512 : (half + 1) * 512],
                        start=(k4 == 0),
                        stop=(k4 == LH * HD // 128 - 1),
                    )
            for half in range(2):
                o_sb = outp.tile([128, 512], mybir.dt.bfloat16, tag="osb", name=f"o_{c}_{half}")
                nc.vector.tensor_copy(out=o_sb[:], in_=po[half][:])
                nc.sync.dma_start(
                    out=out[c * 128 : (c + 1) * 128, half * 512 : (half + 1) * 512],
                    in_=o_sb[:],
                )

        # items: (deadline_g, seq, earliest_g, cost_ns, fn); deadline/earliest
        # in kc-slot units g = u*16 + kc
        fill = []
        seq = 0
        for p in range(LH // 2):
            for tt in range(QT):
                e = max(arr_x[tt], arr_wqk[p])
                fill.append((p * 16 + 4 * tt, seq, e, 1750,
                             (lambda m=2 * p + 1, t=tt: emit_qk_chain(m, t))))
                seq += 1
                fill.append(((4 * tt + p) * 16, seq, e, 1750,
                             (lambda m=2 * p, t=tt: emit_qk_chain(m, t))))
                seq += 1
        for c in range(TOK):
            fill.append((max(6 + c, arr_x[c // 4] + 1), seq,
                         max(arr_x[c // 4], ARR_WV), 1750,
                         (lambda cc=c: emit_v_chain(cc))))
            seq += 1
        fill.sort()
        from collections import deque

        fq = deque(fill)

        SLOT_NS = 1150.0  # ACT pace per kc
        debt = [0.0]

        def flush(g):
            while fq and fq[0][0] <= g:
                _, _, _, cns, fn = fq.popleft()
                fn()
                debt[0] += cns

        def pull(g):
            # pop DMA-ready items while the PE has slack
            while debt[0] < 300.0 and fq:
                if fq[0][2] > g:
                    break
                _, _, _, cns, fn = fq.popleft()
                fn()
                debt[0] += cns
            if debt[0] < -20000.0:
                debt[0] = -20000.0

        # --- the pipelined attention stream --------------------------------
        E_t = [None] * NU
        pw_t = [None] * NU

        def emit_norm(u):
            """Normalize unit u's AV output into waT (no ACT work): gather the
            two denominator rows, one rank-1 broadcast matmul into a scores-
            ring PSUM tile, one DVE reciprocal, two DVE muls."""
            q4u, pu = divmod(u, 4)
            q0 = q4u * 512
            den2 = small.tile([1, 1024], mybir.dt.bfloat16, tag="den", name=f"den_{u}")
            nc.vector.tensor_copy(out=den2[0:1, 0:512], in_=pw_t[u][0][64:65, :])
            nc.vector.tensor_copy(out=den2[0:1, 512:1024], in_=pw_t[u][1][64:65, :])
            pb = ps_m.tile([128, 512], mybir.dt.float32, tag="misc", name=f"pb_{u}")
            for h in range(2):
                nc.tensor.matmul(
                    pb[64 * h : 64 * h + 64, :],
                    lhsT=ones_bf[0:1, 0:64],
                    rhs=den2[0:1, h * 512 : (h + 1) * 512],
                    start=True,
                    stop=True,
                )
            rbr = small.tile([128, 512], mybir.dt.float32, tag="rbr", name=f"rbr_{u}")
            nc.vector.reciprocal_approx_fast(out=rbr[:], in_=pb[:])
            for h01 in range(2):
                nc.vector.tensor_mul(
                    out=waT_sb[64 * h01 : 64 * h01 + 64, pu, q0 : q0 + 512],
                    in0=pw_t[u][h01][0:64, :],
                    in1=rbr[64 * h01 : 64 * h01 + 64, :],
                )
            debt[0] += 700.0

        def emit_av(u, kc):
            q4u, pu = divmod(u, 4)
            if kc == 0:
                pw_t[u] = [
                    ps_wa.tile([65, 512], mybir.dt.float32, tag="wa", name=f"wa_{u}_{h}")
                    for h in range(2)
                ]
            for h01 in range(2):
                nc.tensor.matmul(
                    pw_t[u][h01][:],
                    lhsT=V_sb[:, kc, 2 * pu + h01, :],
                    rhs=E_t[u][:, kc, h01 * 512 : (h01 + 1) * 512],
                    start=(kc == 0),
                    stop=(kc == KC - 1),
                )

        for u in range(NU):
            q4, pair = divmod(u, 4)
            q0 = q4 * 512
            E_t[u] = epool.tile([128, KC, 1024], mybir.dt.bfloat16, tag="E", name=f"E_{u}")
            for ks in range(8):  # 2-kc slots: batch same-shape matmuls
                g = u * 16 + 2 * ks
                if u == NU - 1 and ks == 0:
                    flush(10 ** 9)  # drain all filler while still overlapped
                else:
                    flush(g + 1)
                psc = []
                for j in range(2):
                    kc = 2 * ks + j
                    p_t = ps_s.tile([128, 1024], mybir.dt.float32, tag="sc", name=f"sc_{u}_{kc}")
                    psc.append(p_t)
                    for h01 in range(2):
                        row = 64 * h01
                        nc.tensor.matmul(
                            p_t[:, h01 * 512 : (h01 + 1) * 512],
                            lhsT=qkT_sb[row : row + 64, 2 * pair + 1, kc * 128 : (kc + 1) * 128],
                            rhs=qkT_sb[row : row + 64, 2 * pair, q0 : q0 + 512],
                            start=True,
                            stop=True,
                        )
                for j in range(2):
                    nc.scalar.activation(
                        out=E_t[u][:, 2 * ks + j, :],
                        in_=psc[j][:],
                        func=mybir.ActivationFunctionType.Exp,
                        scale=0.125,
                    )
                if u > 0 and ks % 2 == 1:
                    # 8-MM same-shape AV runs on alternating slots: fewer
                    # PE config transitions than 4-MM runs every slot
                    for kc in range(2 * ks - 2, 2 * ks + 2):
                        emit_av(u - 1, kc)
                if u == NU - 1 and ks > 0:  # drain last unit with 1-slot lag
                    emit_av(u, 2 * (ks - 1))
                    emit_av(u, 2 * (ks - 1) + 1)
                debt[0] += (640.0 if u > 0 else 215.0) * 2 - 2 * SLOT_NS
                flush(g + 3)
                pull(g)
            if u > 0:
                emit_norm(u - 1)
                if (u - 1) % 4 == 3:
                    oq4 = (u - 1) // 4
                    for cc in range(4):
                        fill_dl = u * 16 + 12 * cc + 2
                        fq.append((fill_dl, 0, 0, 1900, (lambda c=oq4 * 4 + cc: emit_out_chain(c))))
                    fq = deque(sorted(fq))

        # drain: last AV slot, final norm, final out-proj; pacer matmuls
        # keep the PE clock warm through the dependency pockets
        emit_av(NU - 1, 14)
        emit_av(NU - 1, 15)
        emit_pacer(4)
        emit_norm(NU - 1)
        while fq:
            fq.popleft()[4]()
        for cc in range(4):
            emit_pacer(3)
            emit_out_chain(12 + cc)

    nc.compile()
    return nc


def _prep_in_maps(x, w_qkv, b_qkv, w_out):
    """Host-side shard + relayout. Core c -> (batch c//2, head-group c%2)."""
    wq = w_qkv[:, :D].reshape(D, H, HD)
    wk = w_qkv[:, D : 2 * D].reshape(D, H, HD)
    wv_ = w_qkv[:, 2 * D :].reshape(D, H, HD)
    bq = b_qkv[:D].reshape(H, HD)
    bk = b_qkv[D : 2 * D].reshape(H, HD)
    wo = w_out.reshape(H, HD, D)

    per_group = {}
    for g in range(G):
        h0 = g * LH
        # qk feature order: chunk 2p = q feats of heads (h0+2p, h0+2p+1)
        # (first head in cols 0-63), chunk 2p+1 = matching k feats.
        Wqk = np.empty((D, DC, 128), F32)
        Bqk = np.empty((DC, 128), F32)
        for p in range(LH // 2):
            ha, hb = h0 + 2 * p, h0 + 2 * p + 1
            Wqk[:, 2 * p, 0:64] = wq[:, ha]
            Wqk[:, 2 * p, 64:128] = wq[:, hb]
            Wqk[:, 2 * p + 1, 0:64] = wk[:, ha]
            Wqk[:, 2 * p + 1, 64:128] = wk[:, hb]
            Bqk[2 * p, 0:64] = bq[ha]
            Bqk[2 * p, 64:128] = bq[hb]
            Bqk[2 * p + 1, 0:64] = bk[ha]
            Bqk[2 * p + 1, 64:128] = bk[hb]
        wqk_arr = np.ascontiguousarray(
            Wqk.reshape(DC, 128, DC * 128).transpose(1, 0, 2)
        ).astype(BF16)
        bqk_arr = np.ascontiguousarray(Bqk.T)

        Wv = wv_[:, h0 : h0 + LH, :].reshape(D, LH * HD)
        wv_arr = np.ascontiguousarray(
            Wv.reshape(DC, 128, LH * HD).transpose(1, 0, 2)
        ).astype(BF16)

        Wo = wo[h0 : h0 + LH].reshape(LH * HD, D)
        wout_arr = np.ascontiguousarray(
            Wo.reshape(LH * HD // 128, 128, D).transpose(1, 0, 2)
        ).astype(BF16)
        per_group[g] = (wqk_arr, bqk_arr, wv_arr, wout_arr)

    in_maps = []
    for c in range(NCORES):
        b, g = divmod(c, G)
        wqk_arr, bqk_arr, wv_arr, wout_arr = per_group[g]
        xT_arr = np.ascontiguousarray(
            x[b].T.reshape(DC, 128, N).transpose(1, 0, 2)
        ).astype(BF16)
        in_maps.append(
            {
                "xT": xT_arr,
                "wqk": wqk_arr,
                "bqk": bqk_arr,
                "wv": wv_arr,
                "wout": wout_arr,
            }
        )
    return in_maps


def _ensure_ntff_hook():
    """Register the axon NTFF profile hook if the image's antenv lacks it."""
    try:
        from antenv.axon_hooks import get_axon_ntff_profile_hook  # noqa: F401

        return
    except ImportError:
        pass

    import contextlib
    import ctypes
    import types

    so_path = "/opt/axon/libaxon_pjrt.so"
    lib = ctypes.CDLL(so_path)
    if not hasattr(lib, "axon_start_nrt_profile"):
        return
    lib.axon_start_nrt_profile.argtypes = [ctypes.POINTER(ctypes.c_int64), ctypes.c_size_t]
    lib.axon_start_nrt_profile.restype = ctypes.c_int64
    lib.axon_stop_nrt_profile.argtypes = [ctypes.c_char_p]
    lib.axon_stop_nrt_profile.restype = ctypes.c_int64

    @contextlib.contextmanager
    def _hook(output_dir, device_ids):
        import jax

        jax.devices()
        if device_ids:
            ids = (ctypes.c_int64 * len(device_ids))(*device_ids)
            rc = lib.axon_start_nrt_profile(ids, len(device_ids))
        else:
            rc = lib.axon_start_nrt_profile(None, 0)
        if rc != 0:
            raise RuntimeError(f"axon_start_nrt_profile rc={rc}")
        try:
            yield
        finally:
            n = lib.axon_stop_nrt_profile(str(output_dir).encode())
            print(f"ntff profile: {n} file(s) written to {output_dir}", file=sys.stderr)

    mod = types.ModuleType("antenv.axon_hooks")
    mod.get_axon_ntff_profile_hook = lambda: _hook
    sys.modules["antenv.axon_hooks"] = mod

    from concourse import bass_utils as _bu

    _bu.upload_artifacts = lambda tmpdir: tmpdir


def kernel(x, w_qkv, b_qkv, w_out, b_out):
    x = np.asarray(x, dtype=F32)
    w_qkv = np.asarray(w_qkv, dtype=F32)
    b_qkv = np.asarray(b_qkv, dtype=F32)
    w_out = np.asarray(w_out, dtype=F32)
    b_out = np.asarray(b_out, dtype=F32)

    if "nc" not in _CACHE:
        _CACHE["nc"] = _build_nc()
    nc = _CACHE["nc"]

    in_maps = _prep_in_maps(x, w_qkv, b_qkv, w_out)
    trace = bool(int(os.environ.get("BASSMHA_TRACE", "0")))
    kwargs = {}
    if trace:
        _ensure_ntff_hook()
        tdir = os.environ.get("BASSMHA_TRACE_DIR")
        if tdir:
            os.makedirs(tdir, exist_ok=True)
            kwargs["tmpdir"] = tdir
    res = run_bass_kernel_spmd(nc, in_maps, list(range(NCORES)), trace=trace, **kwargs)
    _CACHE["last_results"] = res

    # v-bias folded out of the device kernel: attention weights sum to 1,
    # so the bias contributes exactly bv @ w_out per token.
    bias_row = b_out + b_qkv[2 * D :].astype(F32) @ w_out
    out = np.empty((B, N, D), F32)
    for b in range(B):
        out[b] = res.results[2 * b]["out"].astype(F32)
        out[b] += res.results[2 * b + 1]["out"].astype(F32)
        out[b] += bias_row
    return out


# revision 14
# speedup vs baseline: 1.0050x; 1.0031x over previous
"""Multi-head self-attention on 8 Trainium2 NeuronCores.

Problem: x[4, 2048, 1024], 16 heads x 64 dims, fused qkv + attention + out-proj.

Sharding (hybrid, per the tensor-parallel hint): core c handles batch b = c//2
and head-group g = c%2 (8 of the 16 heads). Each core computes a partial
out-projection over its 8 heads; the host sums the two group partials per
batch and adds b_out (+ the folded v-bias term, see below).

The kernel is ACT(exp)-bound: 256 exp tiles of [128, 1024] at ~1.15us each
(~294us of scalar-engine work). Everything else is scheduled around keeping
ACT saturated from ~5us onward:
  - scores computed transposed (S^T[k, q]) per 128-row k-chunk; two heads of
    a pair row-packed on the PE (K=64 each) into one [128, 1024] PSUM tile
    that a single exp covers; exp reads PSUM, writes bf16 E to SBUF.
  - softmax denominator comes free as an all-ones column appended to V in
    the AV matmul (row 64 of the PSUM result).
  - normalization without ACT: PE rank-1 broadcast of the bf16 denominators,
    one DVE reciprocal_approx_fast on the [128, 512] broadcast (reads PSUM),
    then two DVE muls -> normalized waT.  No Ln/Exp table work.
  - v-bias folded out on the host: softmax weights sum to 1 exactly (the
    denominator IS the ones-column sum), so the bias contributes bv @ w_out,
    added to b_out host-side.  q/k biases stay in the projection.
  - emission is software-pipelined: the scores->exp->AV stream is the
    backbone (unit u's AV emitted during unit u+1); qkproj / vproj /
    out-proj chains are deadline-scheduled filler inside the stream, and
    input DMAs are sliced so the first scores matmul can start ~4us in.
"""

import os
import sys
from contextlib import ExitStack

import numpy as np

for _p in ("/opt/trn_rl_repo",):
    if _p not in sys.path and os.path.isdir(_p):
        sys.path.insert(0, _p)

import ml_dtypes

import concourse.bass as bass
import concourse.tile as tile
from concourse import bacc, mybir
from concourse.bass_utils import run_bass_kernel_spmd

BF16 = ml_dtypes.bfloat16
F32 = np.float32

D = 1024
H = 16
HD = 64
B = 4
N = 2048
NCORES = 8
G = 2  # head groups (tensor-parallel axis)
LH = H // G  # local heads per core
DC = D // 128  # 8 contraction chunks
KC = N // 128  # 16 k-token chunks
QT = N // 512  # 4 q tiles
TOK = N // 128  # 16 token chunks
NU = 16  # pipeline units: u = q4*4 + pair

_CACHE = {}


def _pin_act_tables():
    """Pin the act-table chooser so exp resolves to one stable set (no
    mid-kernel table reloads)."""
    if _CACHE.get("act_pinned"):
        return
    from concourse import bacc as _bacc
    from concourse import hw_specs as _hw

    orig = _hw.get_activation_tables

    def patched(arch):
        t = dict(orig(arch))
        keep = "natural_log_exp_and_others"
        if keep in t:
            pinned = t[keep]
            t = {n: (s if n == keep else (s - pinned)) for n, s in t.items()}
        return t

    _hw.get_activation_tables = patched
    _bacc.get_activation_tables = patched
    _CACHE["act_pinned"] = True


def _build_nc():
    _pin_act_tables()
    nc = bacc.Bacc(None, target_bir_lowering=False)

    xT = nc.declare_dram_parameter("xT", [128, DC, N], mybir.dt.bfloat16, isOutput=False)
    wqk = nc.declare_dram_parameter("wqk", [128, DC, 2 * LH * HD], mybir.dt.bfloat16, isOutput=False)
    bqk = nc.declare_dram_parameter("bqk", [128, DC], mybir.dt.float32, isOutput=False)
    wv = nc.declare_dram_parameter("wv", [128, DC, LH * HD], mybir.dt.bfloat16, isOutput=False)
    wout = nc.declare_dram_parameter("wout", [128, LH * HD // 128, D], mybir.dt.bfloat16, isOutput=False)
    out = nc.declare_dram_parameter("out", [N, D], mybir.dt.bfloat16, isOutput=True)

    with tile.TileContext(nc) as tc, ExitStack() as ctx:
        const = ctx.enter_context(tc.tile_pool(name="const", bufs=1))
        xpool = ctx.enter_context(tc.tile_pool(name="xpool", bufs=1))
        epool = ctx.enter_context(tc.tile_pool(name="epool", bufs=2))
        work = ctx.enter_context(tc.tile_pool(name="work", bufs=1))
        outp = ctx.enter_context(tc.tile_pool(name="outp", bufs=2))
        small = ctx.enter_context(tc.tile_pool(name="small", bufs=2))
        ps_s = ctx.enter_context(tc.tile_pool(name="ps_s", bufs=2, space="PSUM"))
        ps_wa = ctx.enter_context(tc.tile_pool(name="ps_wa", bufs=2, space="PSUM"))
        ps_m = ctx.enter_context(tc.tile_pool(name="ps_m", bufs=2, space="PSUM"))

        wqk_sb = const.tile([128, DC, 2 * LH * HD], mybir.dt.bfloat16)
        bqk_sb = const.tile([128, DC], mybir.dt.float32)
        wv_sb = const.tile([128, DC, LH * HD], mybir.dt.bfloat16)
        wout_sb = const.tile([128, LH * HD // 128, D], mybir.dt.bfloat16)
        ones_bf = const.tile([1, 128], mybir.dt.bfloat16)
        xT_sb = xpool.tile([128, DC, N], mybir.dt.bfloat16, tag="xT")
        qkT_sb = work.tile([128, DC, N], mybir.dt.bfloat16, tag="qkT")
        V_sb = work.tile([128, KC, LH, HD + 1], mybir.dt.bfloat16, tag="V")
        waT_sb = work.tile([128, LH * HD // 128, N], mybir.dt.bfloat16, tag="waT")

        # --- DMA: sliced so the first qk chains can start early ------------
        def dma_x(tt, eng, chunked=False):
            # tt0 per contraction-chunk (warm-up pacers key off each chunk);
            # later slices as one descriptor to cut issue serialization
            if chunked:
                for kc in range(DC):
                    eng.dma_start(
                        out=xT_sb[:, kc, tt * 512 : (tt + 1) * 512],
                        in_=xT[:, kc, tt * 512 : (tt + 1) * 512],
                    )
            else:
                eng.dma_start(
                    out=xT_sb[:, :, tt * 512 : (tt + 1) * 512],
                    in_=xT[:, :, tt * 512 : (tt + 1) * 512],
                )

        def dma_wqk(p, eng):
            eng.dma_start(
                out=wqk_sb[:, :, p * 256 : (p + 1) * 256],
                in_=wqk[:, :, p * 256 : (p + 1) * 256],
            )

        # two hardware DGE queues (Sync + Activation) load in parallel; the
        # ACT queue's issue work happens while ACT is otherwise idle in the
        # lead-in.
        dma_wqk(0, nc.sync)
        nc.scalar.dma_start(out=bqk_sb[:], in_=bqk[:])
        dma_x(0, nc.sync, chunked=True)
        nc.scalar.dma_start(out=wv_sb[:], in_=wv[:])
        dma_x(1, nc.scalar)
        dma_x(2, nc.sync)
        dma_x(3, nc.scalar)
        dma_wqk(1, nc.sync)
        dma_wqk(2, nc.scalar)
        dma_wqk(3, nc.sync)
        nc.scalar.dma_start(out=wout_sb[:], in_=wout[:])

        # earliest slot (in kc units, ~1.15us each) each DMA'd tensor is
        # usable; gates the budget puller so a DMA-blocked chain never
        # head-of-line-stalls the PE queue.
        arr_x = [0, 2, 5, 5]
        arr_wqk = [0, 7, 7, 8]
        ARR_WV = 1

        nc.vector.memset(ones_bf[:], 1.0)
        # only the ones column (index HD) of V needs initializing -- it feeds
        # the free softmax denominator; v-proj fills [0:HD].
        nc.vector.memset(V_sb[:, :, :, HD : HD + 1], 1.0)

        # HAM warm-up: a burst of junk matmuls unthrottles the PE clock
        # (~3.4us busy window), then one pacer MM per arriving x-chunk keeps
        # it warm across the DMA lead-in. Results go to a scratch PSUM tile
        # in the scores ring; nothing reads them.
        warm_ps = ps_s.tile([128, 1024], mybir.dt.float32, tag="sc", name="warm_ps")
        for i in range(110):
            nc.tensor.matmul(
                warm_ps[0:128, 0:64],
                lhsT=ones_bf[0:1, 0:128],
                rhs=ones_bf[0:1, 0:64],
                start=True,
                stop=True,
            )

        def emit_pacer(n):
            # junk matmuls that fill PE idle pockets so HAM stays at K=8/8
            for i in range(n):
                nc.tensor.matmul(
                    warm_ps[0:128, 0:128],
                    lhsT=ones_bf[0:1, 0:128],
                    rhs=ones_bf[0:1, 0:128],
                    start=True,
                    stop=True,
                )
        for kc in range(DC):
            nc.tensor.matmul(
                warm_ps[0:64, 0:64],
                lhsT=xT_sb[:, kc, 0:64],
                rhs=xT_sb[:, kc, 0:64],
                start=True,
                stop=True,
            )

        # --- filler work items (deadline-scheduled PE chains) --------------
        def emit_qk_chain(m, tt):
            pq = ps_m.tile([128, 512], mybir.dt.float32, tag="misc", name=f"pq_{m}_{tt}")
            for kc in range(DC):
                nc.tensor.matmul(
                    pq[:],
                    lhsT=wqk_sb[:, kc, m * 128 : (m + 1) * 128],
                    rhs=xT_sb[:, kc, tt * 512 : (tt + 1) * 512],
                    start=(kc == 0),
                    stop=(kc == DC - 1),
                )
            nc.vector.tensor_scalar_add(
                out=qkT_sb[:, m, tt * 512 : (tt + 1) * 512],
                in0=pq[:],
                scalar1=bqk_sb[:, m : m + 1],
            )

        def emit_v_chain(c):
            pv = ps_m.tile([128, 512], mybir.dt.float32, tag="misc", name=f"pv_{c}")
            for kc in range(DC):
                nc.tensor.matmul(
                    pv[:],
                    lhsT=xT_sb[:, kc, c * 128 : (c + 1) * 128],
                    rhs=wv_sb[:, kc, :],
                    start=(kc == 0),
                    stop=(kc == DC - 1),
                )
            nc.vector.tensor_copy(
                out=V_sb[:, c, :, 0:HD],
                in_=pv[:].rearrange("p (l d) -> p l d", l=LH),
            )

        def emit_out_chain(c):
            # k4-outer with both half-D accumulators open: 8 same-shape MMs
            # back-to-back (each lhsT used for both halves)
            po = [
                ps_m.tile([128, 512], mybir.dt.float32, tag="misc", name=f"po_{c}_{h}")
                for h in range(2)
            ]
            for k4 in range(LH * HD // 128):
                for half in range(2):
                    nc.tensor.matmul(
                        po[half][:],
                        lhsT=waT_sb[:, k4, c * 128 : (c + 1) * 128],
                        rhs=wout_sb[:, k4, half * 512 : (half + 1) * 512],
                        start=(k4 == 0),
                        stop=(k4 == LH * HD // 128 - 1),
                    )
            for half in range(2):
                o_sb = outp.tile([128, 512], mybir.dt.bfloat16, tag="osb", name=f"o_{c}_{half}")
                nc.vector.tensor_copy(out=o_sb[:], in_=po[half][:])
                nc.sync.dma_start(
                    out=out[c * 128 : (c + 1) * 128, half * 512 : (half + 1) * 512],
                    in_=o_sb[:],
                )

        # items: (deadline_g, seq, earliest_g, cost_ns, fn); deadline/earliest
        # in kc-slot units g = u*16 + kc
        fill = []
        seq = 0
        for p in range(LH // 2):
            for tt in range(QT):
                e = max(arr_x[tt], arr_wqk[p])
                fill.append((p * 16 + 4 * tt, seq, e, 1750,
                             (lambda m=2 * p + 1, t=tt: emit_qk_chain(m, t))))
                seq += 1
                fill.append(((4 * tt + p) * 16, seq, e, 1750,
                             (lambda m=2 * p, t=tt: emit_qk_chain(m, t))))
                seq += 1
        for c in range(TOK):
            fill.append((max(6 + c, arr_x[c // 4] + 1), seq,
                         max(arr_x[c // 4], ARR_WV), 1750,
                         (lambda cc=c: emit_v_chain(cc))))
            seq += 1
        fill.sort()
        from collections import deque

        fq = deque(fill)

        SLOT_NS = 1150.0  # ACT pace per kc
        debt = [0.0]

        def flush(g):
            while fq and fq[0][0] <= g:
                _, _, _, cns, fn = fq.popleft()
                fn()
                debt[0] += cns

        def pull(g):
            # pop DMA-ready items while the PE has slack
            while debt[0] < 300.0 and fq:
                if fq[0][2] > g:
                    break
                _, _, _, cns, fn = fq.popleft()
                fn()
                debt[0] += cns
            if debt[0] < -20000.0:
                debt[0] = -20000.0

        # --- the pipelined attention stream --------------------------------
        E_t = [None] * NU
        pw_t = [None] * NU

        def emit_norm(u):
            """Normalize unit u's AV output into waT (no ACT work): gather the
            two denominator rows, one rank-1 broadcast matmul into a scores-
            ring PSUM tile, one DVE reciprocal, two DVE muls."""
            q4u, pu = divmod(u, 4)
            q0 = q4u * 512
            den2 = small.tile([1, 1024], mybir.dt.bfloat16, tag="den", name=f"den_{u}")
            nc.vector.tensor_copy(out=den2[0:1, 0:512], in_=pw_t[u][0][64:65, :])
            nc.vector.tensor_copy(out=den2[0:1, 512:1024], in_=pw_t[u][1][64:65, :])
            pb = ps_m.tile([128, 512], mybir.dt.float32, tag="misc", name=f"pb_{u}")
            for h in range(2):
                nc.tensor.matmul(
                    pb[64 * h : 64 * h + 64, :],
                    lhsT=ones_bf[0:1, 0:64],
                    rhs=den2[0:1, h * 512 : (h + 1) * 512],
                    start=True,
                    stop=True,
                )
            rbr = small.tile([128, 512], mybir.dt.float32, tag="rbr", name=f"rbr_{u}")
            nc.vector.reciprocal_approx_fast(out=rbr[:], in_=pb[:])
            for h01 in range(2):
                nc.vector.tensor_mul(
                    out=waT_sb[64 * h01 : 64 * h01 + 64, pu, q0 : q0 + 512],
                    in0=pw_t[u][h01][0:64, :],
                    in1=rbr[64 * h01 : 64 * h01 + 64, :],
                )
            debt[0] += 700.0

        def emit_av(u, kc):
            q4u, pu = divmod(u, 4)
            if kc == 0:
                pw_t[u] = [
                    ps_wa.tile([65, 512], mybir.dt.float32, tag="wa", name=f"wa_{u}_{h}")
                    for h in range(2)
                ]
            for h01 in range(2):
                nc.tensor.matmul(
                    pw_t[u][h01][:],
                    lhsT=V_sb[:, kc, 2 * pu + h01, :],
                    rhs=E_t[u][:, kc, h01 * 512 : (h01 + 1) * 512],
                    start=(kc == 0),
                    stop=(kc == KC - 1),
                )

        for u in range(NU):
            q4, pair = divmod(u, 4)
            q0 = q4 * 512
            E_t[u] = epool.tile([128, KC, 1024], mybir.dt.bfloat16, tag="E", name=f"E_{u}")
            for ks in range(8):  # 2-kc slots: batch same-shape matmuls
                g = u * 16 + 2 * ks
                if u == NU - 1 and ks == 0:
                    flush(10 ** 9)  # drain all filler while still overlapped
                else:
                    flush(g + 1)
                psc = []
                for j in range(2):
                    kc = 2 * ks + j
                    p_t = ps_s.tile([128, 1024], mybir.dt.float32, tag="sc", name=f"sc_{u}_{kc}")
                    psc.append(p_t)
                    for h01 in range(2):
                        row = 64 * h01
                        nc.tensor.matmul(
                            p_t[:, h01 * 512 : (h01 + 1) * 512],
                            lhsT=qkT_sb[row : row + 64, 2 * pair + 1, kc * 128 : (kc + 1) * 128],
                            rhs=qkT_sb[row : row + 64, 2 * pair, q0 : q0 + 512],
                            start=True,
                            stop=True,
                        )
                for j in range(2):
                    nc.scalar.activation(
                        out=E_t[u][:, 2 * ks + j, :],
                        in_=psc[j][:],
                        func=mybir.ActivationFunctionType.Exp,
                        scale=0.125,
                    )
                if u > 0 and ks % 2 == 1:
                    # 8-MM same-shape AV runs on alternating slots: fewer
                    # PE config transitions than 4-MM runs every slot
                    for kc in range(2 * ks - 2, 2 * ks + 2):
                        emit_av(u - 1, kc)
                if u == NU - 1 and ks > 0:  # drain last unit with 1-slot lag
                    emit_av(u, 2 * (ks - 1))
                    emit_av(u, 2 * (ks - 1) + 1)
                debt[0] += (640.0 if u > 0 else 215.0) * 2 - 2 * SLOT_NS
                flush(g + 3)
                pull(g)
            if u > 0:
                emit_norm(u - 1)
                if (u - 1) % 4 == 3:
                    oq4 = (u - 1) // 4
                    for cc in range(4):
                        fill_dl = u * 16 + 12 * cc + 2
                        fq.append((fill_dl, 0, 0, 1900, (lambda c=oq4 * 4 + cc: emit_out_chain(c))))
                    fq = deque(sorted(fq))

        # drain: last AV slot, final norm, final out-proj; pacer matmuls
        # keep the PE clock warm through the dependency pockets
        emit_av(NU - 1, 14)
        emit_av(NU - 1, 15)
        emit_pacer(4)
        emit_norm(NU - 1)
        while fq:
            fq.popleft()[4]()
        for cc in range(4):
            emit_pacer(3)
            emit_out_chain(12 + cc)

    nc.compile()
    return nc


def _prep_in_maps(x, w_qkv, b_qkv, w_out):
    """Host-side shard + relayout. Core c -> (batch c//2, head-group c%2)."""
    wq = w_qkv[:, :D].reshape(D, H, HD)
    wk = w_qkv[:, D : 2 * D].reshape(D, H, HD)
    wv_ = w_qkv[:, 2 * D :].reshape(D, H, HD)
    bq = b_qkv[:D].reshape(H, HD)
    bk = b_qkv[D : 2 * D].reshape(H, HD)
    wo = w_out.reshape(H, HD, D)

    per_group = {}
    for g in range(G):
        h0 = g * LH
        # qk feature order: chunk 2p = q feats of heads (h0+2p, h0+2p+1)
        # (first head in cols 0-63), chunk 2p+1 = matching k feats.
        Wqk = np.empty((D, DC, 128), F32)
        Bqk = np.empty((DC, 128), F32)
        for p in range(LH // 2):
            ha, hb = h0 + 2 * p, h0 + 2 * p + 1
            Wqk[:, 2 * p, 0:64] = wq[:, ha]
            Wqk[:, 2 * p, 64:128] = wq[:, hb]
            Wqk[:, 2 * p + 1, 0:64] = wk[:, ha]
            Wqk[:, 2 * p + 1, 64:128] = wk[:, hb]
            Bqk[2 * p, 0:64] = bq[ha]
            Bqk[2 * p, 64:128] = bq[hb]
            Bqk[2 * p + 1, 0:64] = bk[ha]
            Bqk[2 * p + 1, 64:128] = bk[hb]
        wqk_arr = np.ascontiguousarray(
            Wqk.reshape(DC, 128, DC * 128).transpose(1, 0, 2)
        ).astype(BF16)
        bqk_arr = np.ascontiguousarray(Bqk.T)

        Wv = wv_[:, h0 : h0 + LH, :].reshape(D, LH * HD)
        wv_arr = np.ascontiguousarray(
            Wv.reshape(DC, 128, LH * HD).transpose(1, 0, 2)
        ).astype(BF16)

        Wo = wo[h0 : h0 + LH].reshape(LH * HD, D)
        wout_arr = np.ascontiguousarray(
            Wo.reshape(LH * HD // 128, 128, D).transpose(1, 0, 2)
        ).astype(BF16)
        per_group[g] = (wqk_arr, bqk_arr, wv_arr, wout_arr)

    in_maps = []
    for c in range(NCORES):
        b, g = divmod(c, G)
        wqk_arr, bqk_arr, wv_arr, wout_arr = per_group[g]
        xT_arr = np.ascontiguousarray(
            x[b].T.reshape(DC, 128, N).transpose(1, 0, 2)
        ).astype(BF16)
        in_maps.append(
            {
                "xT": xT_arr,
                "wqk": wqk_arr,
                "bqk": bqk_arr,
                "wv": wv_arr,
                "wout": wout_arr,
            }
        )
    return in_maps


def _ensure_ntff_hook():
    """Register the axon NTFF profile hook if the image's antenv lacks it."""
    try:
        from antenv.axon_hooks import get_axon_ntff_profile_hook  # noqa: F401

        return
    except ImportError:
        pass

    import contextlib
    import ctypes
    import types

    so_path = "/opt/axon/libaxon_pjrt.so"
    lib = ctypes.CDLL(so_path)
    if not hasattr(lib, "axon_start_nrt_profile"):
        return
    lib.axon_start_nrt_profile.argtypes = [ctypes.POINTER(ctypes.c_int64), ctypes.c_size_t]
    lib.axon_start_nrt_profile.restype = ctypes.c_int64
    lib.axon_stop_nrt_profile.argtypes = [ctypes.c_char_p]
    lib.axon_stop_nrt_profile.restype = ctypes.c_int64

    @contextlib.contextmanager
    def _hook(output_dir, device_ids):
        import jax

        jax.devices()
        if device_ids:
            ids = (ctypes.c_int64 * len(device_ids))(*device_ids)
            rc = lib.axon_start_nrt_profile(ids, len(device_ids))
        else:
            rc = lib.axon_start_nrt_profile(None, 0)
        if rc != 0:
            raise RuntimeError(f"axon_start_nrt_profile rc={rc}")
        try:
            yield
        finally:
            n = lib.axon_stop_nrt_profile(str(output_dir).encode())
            print(f"ntff profile: {n} file(s) written to {output_dir}", file=sys.stderr)

    mod = types.ModuleType("antenv.axon_hooks")
    mod.get_axon_ntff_profile_hook = lambda: _hook
    sys.modules["antenv.axon_hooks"] = mod

    from concourse import bass_utils as _bu

    _bu.upload_artifacts = lambda tmpdir: tmpdir


def kernel(x, w_qkv, b_qkv, w_out, b_out):
    x = np.asarray(x, dtype=F32)
    w_qkv = np.asarray(w_qkv, dtype=F32)
    b_qkv = np.asarray(b_qkv, dtype=F32)
    w_out = np.asarray(w_out, dtype=F32)
    b_out = np.asarray(b_out, dtype=F32)

    if "nc" not in _CACHE:
        _CACHE["nc"] = _build_nc()
    nc = _CACHE["nc"]

    in_maps = _prep_in_maps(x, w_qkv, b_qkv, w_out)
    trace = bool(int(os.environ.get("BASSMHA_TRACE", "0")))
    kwargs = {}
    if trace:
        _ensure_ntff_hook()
        tdir = os.environ.get("BASSMHA_TRACE_DIR")
        if tdir:
            os.makedirs(tdir, exist_ok=True)
            kwargs["tmpdir"] = tdir
    res = run_bass_kernel_spmd(nc, in_maps, list(range(NCORES)), trace=trace, **kwargs)
    _CACHE["last_results"] = res

    # v-bias folded out of the device kernel: attention weights sum to 1,
    # so the bias contributes exactly bv @ w_out per token.
    bias_row = b_out + b_qkv[2 * D :].astype(F32) @ w_out
    out = np.empty((B, N, D), F32)
    for b in range(B):
        out[b] = res.results[2 * b]["out"].astype(F32)
        out[b] += res.results[2 * b + 1]["out"].astype(F32)
        out[b] += bias_row
    return out
